# revision 2
# baseline (speedup 1.0000x reference)
"""Trainium2 Bass kernel for nn_MCPInitEmbedding (segment_reduce).

Problem: out[b,s,:] = sum_{j<100} (weights[b, idx[b,s,j]] * w + bias)
                   = S[b,s] * w + 100*bias,   S = C @ wtab
where C[s,i] is the multiplicity of item i in set s (sum_i C[s,i] = 100).

Device algorithm (per core = 2 batches, pure data parallel over 8 cores):
split item index i = q*128 + r. For each (batch, 128-set block), run 79
accumulating GEMVs on the tensor engine:
    psum[s] += Cq[r, s]^T @ T[r, q]        (lhsT = count block, fp8)
with fast-weight-load streaming the fp8 count blocks through the PE at
~512B/cycle. The rank-1 expansion out[s,:] = S[s]*w + 100*b is one
scalar_tensor_tensor on the DVE per block (per-partition scalar read
straight from PSUM), then a direct [128 sets x 128 D] store.

Count blocks ship BIT-PACKED (4 counts x 2 bits per byte when all counts
<= 3, else 2 x 4 bits when <= 7). The DVE unpacks with one shift+mask per
bit-plane on uint16 lanes; the resulting bytes are integers c in 0..7,
whose fp8e4m3 bit patterns are DENORMALS with value exactly c * 2^-9
(denormal mantissas are linear - verified exact on hardware). The 512x
rescale is folded into the expansion's w row. Counts > 7 (never for
realistic inputs) fall back to unpacked host-built fp8 values.

The packed count matrix is ~5MB/core; DMA, DVE unpack, PE GEMVs and the
expansion/store pipeline across chunks (PE is the critical path at
~34ns/GEMV). Measured on trn2: ~59us vs 717us for the gpsimd ap_gather
baseline (~12x).
"""
import numpy as np
import ml_dtypes

import concourse.bacc as bacc
import concourse.tile as tile
import concourse.mybir as mybir
from concourse.bass_utils import run_bass_kernel_spmd

B = 16
N_ITEMS = 10000
N_SETS = 1000
SET_SZ = 100
D = 128
N_CORES = 8
BPC = B // N_CORES  # batches per core

Q = 79            # ceil(N_ITEMS / 128) q-planes
QP = 80           # padded to a multiple of 4
NBLK = 8          # 1000 sets -> 8 blocks of 128 (last holds 104)

F32 = mybir.dt.float32
F16 = mybir.dt.float16
F8 = mybir.dt.float8e4
U16 = mybir.dt.uint16

_CACHED = {}


def _build_program(nj):
    """nj = counts packed per byte (4 -> 2-bit fields, 2 -> 4-bit, 1 -> raw
    fp8 value bytes built on host)."""
    nqq = QP // nj                # byte-planes per chunk
    pk_b = nqq * 128              # packed bytes/partition/(batch, block)
    pk_w = pk_b // 2              # as uint16 words
    bits = 8 // nj
    mask = (1 << bits) - 1
    mask16 = mask | (mask << 8)

    nc = bacc.Bacc("TRN2", target_bir_lowering=False, debug=False,
                   num_devices=N_CORES)
    cpk = nc.dram_tensor("cpk", [128, BPC * NBLK * pk_w], U16,
                         kind="ExternalInput").ap()
    wt16 = nc.dram_tensor("wt16", [128, BPC * Q], F16,
                          kind="ExternalInput").ap()
    wbrow = nc.dram_tensor("wbrow", [128, 2 * D], F32,
                           kind="ExternalInput").ap()
    out = nc.dram_tensor("out", [BPC, N_SETS, D], F32,
                         kind="ExternalOutput").ap()

    with tile.TileContext(nc) as tc:
        with (
            tc.tile_pool(name="main", bufs=1) as pool,
            tc.tile_pool(name="pk", bufs=6) as pkpool,
            tc.tile_pool(name="up", bufs=3 * nj) as uppool,
            tc.tile_pool(name="ob", bufs=2) as opool,
            tc.tile_pool(name="ps", bufs=4, space="PSUM") as psp,
        ):
            wtt = pool.tile([128, BPC * Q], F16)
            wbt = pool.tile([128, 2 * D], F32)
            nc.scalar.dma_start(wtt[:], wt16)
            nc.scalar.dma_start(wbt[:], wbrow)

            def _expand_store(ps, bb, blk):
                # out[s, :] = S[s] * w + 100*b; S read straight from PSUM
                ob = opool.tile([128, D], F32, tag="ob")
                nc.vector.scalar_tensor_tensor(
                    ob[:],
                    wbt[:, :D],
                    ps[:],
                    wbt[:, D : 2 * D],
                    op0=mybir.AluOpType.mult,
                    op1=mybir.AluOpType.add,
                )
                ns = min(128, N_SETS - blk * 128)
                nc.scalar.dma_start(
                    out[bb, blk * 128 : blk * 128 + ns, :], ob[:ns, :]
                )

            pending = None
            for bb in range(BPC):
                for blk in range(NBLK):
                    ci = bb * NBLK + blk
                    pkt = pkpool.tile([128, pk_w], U16, tag="pk")
                    nc.sync.dma_start(
                        pkt[:], cpk[:, ci * pk_w : (ci + 1) * pk_w]
                    )
                    upj = []
                    if nj == 1:
                        upj.append(pkt[:].bitcast(F8))
                    else:
                        for j in range(nj):
                            u = uppool.tile([128, pk_w], U16, tag="up")
                            nc.vector.tensor_scalar(
                                u[:],
                                pkt[:],
                                bits * j,
                                mask16,
                                op0=mybir.AluOpType.logical_shift_right,
                                op1=mybir.AluOpType.bitwise_and,
                            )
                            upj.append(u[:].bitcast(F8))
                    ps = psp.tile([128, 1], F32, tag="ps")
                    for q in range(Q):
                        j, qq = q // nqq, q % nqq
                        nc.tensor.matmul(
                            out=ps[:],
                            lhsT=upj[j][:, qq * 128 : (qq + 1) * 128],
                            rhs=wtt[:, bb * Q + q : bb * Q + q + 1],
                            start=(q == 0),
                            stop=(q == Q - 1),
                        )
                    # defer the expansion one iteration so the DVE unpacks
                    # the next chunk before turning the PSUM column around
                    if pending is not None:
                        _expand_store(*pending)
                    pending = (ps, bb, blk)
            _expand_store(*pending)

    nc.compile()
    return nc


def _counts(mem_core):
    """Count tensor A[r, bb, blk, q(80), m] for one core's batches."""
    A = np.zeros((128, BPC, NBLK, QP, 128), dtype=np.uint8)
    idx = mem_core.astype(np.int64)
    bb_ix = np.broadcast_to(np.arange(BPC)[:, None, None], idx.shape).reshape(-1)
    s_ix = np.broadcast_to(np.arange(N_SETS)[None, :, None], idx.shape).reshape(-1)
    i_ix = idx.reshape(-1)
    np.add.at(A, (i_ix & 127, bb_ix, s_ix >> 7, i_ix >> 7, s_ix & 127), 1)
    return A


def _pack(A, nj):
    """Pack nj counts per byte; plane j holds q = j*(QP//nj) + qq."""
    if nj == 1:
        # raw fp8 value bytes (exact for counts <= 448); scale 1.0
        P = A.astype(ml_dtypes.float8_e4m3).view(np.uint8)
        scale = 1.0
    else:
        bits = 8 // nj
        nqq = QP // nj
        Aj = A.reshape(128, BPC, NBLK, nj, nqq, 128)
        P = Aj[:, :, :, 0].astype(np.uint8).copy()
        for j in range(1, nj):
            P |= Aj[:, :, :, j] << (bits * j)
        scale = 512.0  # denormal ints decode as c * 2^-9
    return np.ascontiguousarray(P.reshape(128, -1)).view(np.uint16), scale


def _build_inputs(weights_core, mem_core, w, b, nj, scale):
    cpk, _ = _pack(_counts(mem_core), nj)

    wt = np.zeros((BPC, Q * 128), dtype=np.float32)
    wt[:, :N_ITEMS] = weights_core
    wt16 = (
        wt.reshape(BPC, Q, 128).transpose(2, 0, 1).reshape(128, BPC * Q)
    ).astype(np.float16)

    wbrow = np.empty((128, 2 * D), dtype=np.float32)
    wbrow[:, :D] = scale * w[None, :]
    wbrow[:, D:] = SET_SZ * b[None, :]
    return {"cpk": cpk, "wt16": wt16, "wbrow": wbrow}


def _choose_nj(membership):
    counts_max = 0
    for c in range(N_CORES):
        counts_max = max(
            counts_max, int(_counts(membership[c * BPC : (c + 1) * BPC]).max())
        )
    if counts_max <= 3:
        return 4
    if counts_max <= 7:
        return 2
    return 1


def kernel(weights, membership, w, b):
    weights = np.asarray(weights, dtype=np.float32)
    membership = np.asarray(membership)
    w = np.asarray(w, dtype=np.float32)
    b = np.asarray(b, dtype=np.float32)

    nj = _choose_nj(membership)
    key = f"nc{nj}"
    if key not in _CACHED:
        _CACHED[key] = _build_program(nj)
    nc = _CACHED[key]
    scale = 512.0 if nj > 1 else 1.0

    in_maps = [
        _build_inputs(
            weights[c * BPC : (c + 1) * BPC],
            membership[c * BPC : (c + 1) * BPC],
            w, b, nj, scale,
        )
        for c in range(N_CORES)
    ]
    res = run_bass_kernel_spmd(nc, in_maps, core_ids=list(range(N_CORES)))
    out = np.concatenate(
        [res.results[c]["out"] for c in range(N_CORES)], axis=0
    )
    return out.astype(np.float32)


# revision 3
# speedup vs baseline: 1.0024x; 1.0024x over previous
"""Trainium2 Bass kernel for nn_MCPInitEmbedding (segment_reduce).

Problem: out[b,s,:] = sum_{j<100} (weights[b, idx[b,s,j]] * w + bias)
                   = S[b,s] * w + 100*bias,   S = C @ wtab
where C[s,i] is the multiplicity of item i in set s (sum_i C[s,i] = 100).

Device algorithm (per core = 2 batches, pure data parallel over 8 cores):
split item index i = q*128 + r. For each (batch, 128-set block), run 79
accumulating GEMVs on the tensor engine:
    psum[s] += Cq[r, s]^T @ T[r, q]        (lhsT = count block, fp8)
with fast-weight-load streaming the fp8 count blocks through the PE at
~512B/cycle. The rank-1 expansion out[s,:] = S[s]*w + 100*b is one
scalar_tensor_tensor on the DVE per block (per-partition scalar read
straight from PSUM), then a direct [128 sets x 128 D] store.

Count blocks ship BIT-PACKED (4 counts x 2 bits per byte when all counts
<= 3, else 2 x 4 bits when <= 7). The DVE unpacks with one shift+mask per
bit-plane on uint16 lanes; the resulting bytes are integers c in 0..7,
whose fp8e4m3 bit patterns are DENORMALS with value exactly c * 2^-9
(denormal mantissas are linear - verified exact on hardware). The 512x
rescale is folded into the expansion's w row. Counts > 7 (never for
realistic inputs) fall back to unpacked host-built fp8 values.

The packed count matrix is ~5MB/core; DMA, DVE unpack, PE GEMVs and the
expansion/store pipeline across chunks (PE is the critical path at
~34ns/GEMV). Measured on trn2: ~59us vs 717us for the gpsimd ap_gather
baseline (~12x).
"""
import numpy as np
import ml_dtypes

import concourse.bacc as bacc
import concourse.tile as tile
import concourse.mybir as mybir
from concourse.bass_utils import run_bass_kernel_spmd

B = 16
N_ITEMS = 10000
N_SETS = 1000
SET_SZ = 100
D = 128
N_CORES = 8
BPC = B // N_CORES  # batches per core

Q = 79            # ceil(N_ITEMS / 128) q-planes
QP = 80           # padded to a multiple of 4
NBLK = 8          # 1000 sets -> 8 blocks of 128 (last holds 104)

F32 = mybir.dt.float32
F16 = mybir.dt.float16
F8 = mybir.dt.float8e4
U16 = mybir.dt.uint16

_CACHED = {}


def _build_program(nj):
    """nj = counts packed per byte (4 -> 2-bit fields, 2 -> 4-bit, 1 -> raw
    fp8 value bytes built on host)."""
    nqq = QP // nj                # byte-planes per chunk
    pk_b = nqq * 128              # packed bytes/partition/(batch, block)
    pk_w = pk_b // 2              # as uint16 words
    bits = 8 // nj
    mask = (1 << bits) - 1
    mask16 = mask | (mask << 8)

    nc = bacc.Bacc("TRN2", target_bir_lowering=False, debug=False,
                   num_devices=N_CORES)
    cpk = nc.dram_tensor("cpk", [128, BPC * NBLK * pk_w], U16,
                         kind="ExternalInput").ap()
    wt16 = nc.dram_tensor("wt16", [128, BPC * Q], F16,
                          kind="ExternalInput").ap()
    wbrow = nc.dram_tensor("wbrow", [128, 2 * D], F32,
                           kind="ExternalInput").ap()
    out = nc.dram_tensor("out", [BPC, N_SETS, D], F32,
                         kind="ExternalOutput").ap()

    with tile.TileContext(nc) as tc:
        with (
            tc.tile_pool(name="main", bufs=1) as pool,
            tc.tile_pool(name="pk", bufs=6) as pkpool,
            tc.tile_pool(name="up", bufs=3 * nj) as uppool,
            tc.tile_pool(name="ob", bufs=2) as opool,
            tc.tile_pool(name="ps", bufs=8, space="PSUM") as psp,
        ):
            wtt = pool.tile([128, BPC * Q], F16)
            wbt = pool.tile([128, 2 * D], F32)
            nc.scalar.dma_start(wtt[:], wt16)
            nc.scalar.dma_start(wbt[:], wbrow)

            def _expand_store(ps, bb, blk):
                # out[s, :] = S[s] * w + 100*b; S read straight from PSUM
                ob = opool.tile([128, D], F32, tag="ob")
                nc.vector.scalar_tensor_tensor(
                    ob[:],
                    wbt[:, :D],
                    ps[:],
                    wbt[:, D : 2 * D],
                    op0=mybir.AluOpType.mult,
                    op1=mybir.AluOpType.add,
                )
                ns = min(128, N_SETS - blk * 128)
                nc.scalar.dma_start(
                    out[bb, blk * 128 : blk * 128 + ns, :], ob[:ns, :]
                )

            pending = None
            for bb in range(BPC):
                for blk in range(NBLK):
                    ci = bb * NBLK + blk
                    pkt = pkpool.tile([128, pk_w], U16, tag="pk")
                    nc.sync.dma_start(
                        pkt[:], cpk[:, ci * pk_w : (ci + 1) * pk_w]
                    )
                    upj = []
                    if nj == 1:
                        upj.append(pkt[:].bitcast(F8))
                    else:
                        for j in range(nj):
                            u = uppool.tile([128, pk_w], U16, tag="up")
                            nc.vector.tensor_scalar(
                                u[:],
                                pkt[:],
                                bits * j,
                                mask16,
                                op0=mybir.AluOpType.logical_shift_right,
                                op1=mybir.AluOpType.bitwise_and,
                            )
                            upj.append(u[:].bitcast(F8))
                    ps = psp.tile([128, 1], F32, tag="ps")
                    for q in range(Q):
                        j, qq = q // nqq, q % nqq
                        nc.tensor.matmul(
                            out=ps[:],
                            lhsT=upj[j][:, qq * 128 : (qq + 1) * 128],
                            rhs=wtt[:, bb * Q + q : bb * Q + q + 1],
                            start=(q == 0),
                            stop=(q == Q - 1),
                        )
                    # defer the expansion one iteration so the DVE unpacks
                    # the next chunk before turning the PSUM column around
                    if pending is not None:
                        _expand_store(*pending)
                    pending = (ps, bb, blk)
            _expand_store(*pending)

    nc.compile()
    return nc


def _counts(mem_core):
    """Count tensor A[r, bb, blk, q(80), m] for one core's batches."""
    A = np.zeros((128, BPC, NBLK, QP, 128), dtype=np.uint8)
    idx = mem_core.astype(np.int64)
    bb_ix = np.broadcast_to(np.arange(BPC)[:, None, None], idx.shape).reshape(-1)
    s_ix = np.broadcast_to(np.arange(N_SETS)[None, :, None], idx.shape).reshape(-1)
    i_ix = idx.reshape(-1)
    np.add.at(A, (i_ix & 127, bb_ix, s_ix >> 7, i_ix >> 7, s_ix & 127), 1)
    return A


def _pack(A, nj):
    """Pack nj counts per byte; plane j holds q = j*(QP//nj) + qq."""
    if nj == 1:
        # raw fp8 value bytes (exact for counts <= 448); scale 1.0
        P = A.astype(ml_dtypes.float8_e4m3).view(np.uint8)
        scale = 1.0
    else:
        bits = 8 // nj
        nqq = QP // nj
        Aj = A.reshape(128, BPC, NBLK, nj, nqq, 128)
        P = Aj[:, :, :, 0].astype(np.uint8).copy()
        for j in range(1, nj):
            P |= Aj[:, :, :, j] << (bits * j)
        scale = 512.0  # denormal ints decode as c * 2^-9
    return np.ascontiguousarray(P.reshape(128, -1)).view(np.uint16), scale


def _build_inputs(weights_core, mem_core, w, b, nj, scale):
    cpk, _ = _pack(_counts(mem_core), nj)

    wt = np.zeros((BPC, Q * 128), dtype=np.float32)
    wt[:, :N_ITEMS] = weights_core
    wt16 = (
        wt.reshape(BPC, Q, 128).transpose(2, 0, 1).reshape(128, BPC * Q)
    ).astype(np.float16)

    wbrow = np.empty((128, 2 * D), dtype=np.float32)
    wbrow[:, :D] = scale * w[None, :]
    wbrow[:, D:] = SET_SZ * b[None, :]
    return {"cpk": cpk, "wt16": wt16, "wbrow": wbrow}


def _choose_nj(membership):
    counts_max = 0
    for c in range(N_CORES):
        counts_max = max(
            counts_max, int(_counts(membership[c * BPC : (c + 1) * BPC]).max())
        )
    if counts_max <= 3:
        return 4
    if counts_max <= 7:
        return 2
    return 1


def kernel(weights, membership, w, b):
    weights = np.asarray(weights, dtype=np.float32)
    membership = np.asarray(membership)
    w = np.asarray(w, dtype=np.float32)
    b = np.asarray(b, dtype=np.float32)

    nj = _choose_nj(membership)
    key = f"nc{nj}"
    if key not in _CACHED:
        _CACHED[key] = _build_program(nj)
    nc = _CACHED[key]
    scale = 512.0 if nj > 1 else 1.0

    in_maps = [
        _build_inputs(
            weights[c * BPC : (c + 1) * BPC],
            membership[c * BPC : (c + 1) * BPC],
            w, b, nj, scale,
        )
        for c in range(N_CORES)
    ]
    res = run_bass_kernel_spmd(nc, in_maps, core_ids=list(range(N_CORES)))
    out = np.concatenate(
        [res.results[c]["out"] for c in range(N_CORES)], axis=0
    )
    return out.astype(np.float32)


# revision 6
# speedup vs baseline: 1.0066x; 1.0042x over previous
"""Trainium2 Bass kernel for nn_MCPInitEmbedding (segment_reduce).

Problem: out[b,s,:] = sum_{j<100} (weights[b, idx[b,s,j]] * w + bias)
                   = S[b,s] * w + 100*bias,   S = C @ wtab
where C[s,i] is the multiplicity of item i in set s (sum_i C[s,i] = 100).

Device algorithm (per core = 2 batches, pure data parallel over 8 cores):
split item index i = q*128 + r. For each (batch, 128-set block), run 79
accumulating GEMVs on the tensor engine:
    psum[s] += Cq[r, s]^T @ T[r, q]        (lhsT = count block, fp8)
with fast-weight-load streaming the fp8 count blocks through the PE at
~512B/cycle. The rank-1 expansion out[s,:] = S[s]*w + 100*b is one
scalar_tensor_tensor on the DVE per block (per-partition scalar read
straight from PSUM), then a direct [128 sets x 128 D] store.

Count blocks ship BIT-PACKED (4 counts x 2 bits per byte when all counts
<= 3, else 2 x 4 bits when <= 7). The DVE unpacks with one shift+mask per
bit-plane on uint16 lanes; the resulting bytes are integers c in 0..7,
whose fp8e4m3 bit patterns are DENORMALS with value exactly c * 2^-9
(denormal mantissas are linear - verified exact on hardware). The 512x
rescale is folded into the expansion's w row. Counts > 7 (never for
realistic inputs) fall back to unpacked host-built fp8 values.

The packed count matrix is ~5MB/core; DMA, DVE unpack, PE GEMVs and the
expansion/store pipeline across chunks (PE is the critical path: 1264
LDWEIGHTS+MATMUL pairs at a fixed ~33.6ns/pair, dtype- and N-independent
for small N). Measured on trn2: 58.8-59.7us across runs vs 717us for the
gpsimd ap_gather baseline (~12x). Breakdown: ~6.4us NEFF/tile-context
init, ~4.6us pipeline fill, ~42.5us PE GEMV stream, ~5us drain/end.
"""
import numpy as np
import ml_dtypes

import concourse.bacc as bacc
import concourse.tile as tile
import concourse.mybir as mybir
from concourse.bass_utils import run_bass_kernel_spmd

B = 16
N_ITEMS = 10000
N_SETS = 1000
SET_SZ = 100
D = 128
N_CORES = 8
BPC = B // N_CORES  # batches per core

Q = 79            # ceil(N_ITEMS / 128) q-planes
QP = 80           # padded to a multiple of 4
NBLK = 8          # 1000 sets -> 8 blocks of 128 (last holds 104)

F32 = mybir.dt.float32
F16 = mybir.dt.float16
F8 = mybir.dt.float8e4
U16 = mybir.dt.uint16

_CACHED = {}


def _build_program(nj):
    """nj = counts packed per byte (4 -> 2-bit fields, 2 -> 4-bit, 1 -> raw
    fp8 value bytes built on host)."""
    nqq = QP // nj                # byte-planes per chunk
    pk_b = nqq * 128              # packed bytes/partition/(batch, block)
    pk_w = pk_b // 2              # as uint16 words
    bits = 8 // nj
    mask = (1 << bits) - 1
    mask16 = mask | (mask << 8)

    nc = bacc.Bacc("TRN2", target_bir_lowering=False, debug=False,
                   num_devices=N_CORES)
    cpk = nc.dram_tensor("cpk", [128, BPC * NBLK * pk_w], U16,
                         kind="ExternalInput").ap()
    wt16 = nc.dram_tensor("wt16", [128, BPC * Q], F16,
                          kind="ExternalInput").ap()
    wbrow = nc.dram_tensor("wbrow", [128, 2 * D], F32,
                           kind="ExternalInput").ap()
    out = nc.dram_tensor("out", [BPC, N_SETS, D], F32,
                         kind="ExternalOutput").ap()

    with tile.TileContext(nc) as tc:
        with (
            tc.tile_pool(name="main", bufs=1) as pool,
            tc.tile_pool(name="pk", bufs=6) as pkpool,
            tc.tile_pool(name="up", bufs=3 * nj) as uppool,
            tc.tile_pool(name="ob", bufs=2) as opool,
            tc.tile_pool(name="ps", bufs=8, space="PSUM") as psp,
        ):
            wtt = pool.tile([128, BPC * Q], F16)
            wbt = pool.tile([128, 2 * D], F32)
            nc.scalar.dma_start(wtt[:], wt16)
            nc.scalar.dma_start(wbt[:], wbrow)

            def _expand_store(ps, bb, blk):
                # out[s, :] = S[s] * w + 100*b; S read straight from PSUM
                ob = opool.tile([128, D], F32, tag="ob")
                nc.vector.scalar_tensor_tensor(
                    ob[:],
                    wbt[:, :D],
                    ps[:],
                    wbt[:, D : 2 * D],
                    op0=mybir.AluOpType.mult,
                    op1=mybir.AluOpType.add,
                )
                ns = min(128, N_SETS - blk * 128)
                nc.scalar.dma_start(
                    out[bb, blk * 128 : blk * 128 + ns, :], ob[:ns, :]
                )

            pending = None
            for bb in range(BPC):
                for blk in range(NBLK):
                    ci = bb * NBLK + blk
                    pkt = pkpool.tile([128, pk_w], U16, tag="pk")
                    nc.sync.dma_start(
                        pkt[:], cpk[:, ci * pk_w : (ci + 1) * pk_w]
                    )
                    upj = []
                    if nj == 1:
                        upj.append(pkt[:].bitcast(F8))
                    else:
                        for j in range(nj):
                            u = uppool.tile([128, pk_w], U16, tag="up")
                            nc.vector.tensor_scalar(
                                u[:],
                                pkt[:],
                                bits * j,
                                mask16,
                                op0=mybir.AluOpType.logical_shift_right,
                                op1=mybir.AluOpType.bitwise_and,
                            )
                            upj.append(u[:].bitcast(F8))
                    ps = psp.tile([128, 1], F32, tag="ps")
                    for q in range(Q):
                        j, qq = q // nqq, q % nqq
                        nc.tensor.matmul(
                            out=ps[:],
                            lhsT=upj[j][:, qq * 128 : (qq + 1) * 128],
                            rhs=wtt[:, bb * Q + q : bb * Q + q + 1],
                            start=(q == 0),
                            stop=(q == Q - 1),
                        )
                    # defer the expansion one iteration so the DVE unpacks
                    # the next chunk before turning the PSUM column around
                    if pending is not None:
                        _expand_store(*pending)
                    pending = (ps, bb, blk)
            _expand_store(*pending)

    nc.compile()
    return nc


def _counts(mem_core):
    """Count tensor A[r, bb, blk, q(80), m] for one core's batches."""
    A = np.zeros((128, BPC, NBLK, QP, 128), dtype=np.uint8)
    idx = mem_core.astype(np.int64)
    bb_ix = np.broadcast_to(np.arange(BPC)[:, None, None], idx.shape).reshape(-1)
    s_ix = np.broadcast_to(np.arange(N_SETS)[None, :, None], idx.shape).reshape(-1)
    i_ix = idx.reshape(-1)
    np.add.at(A, (i_ix & 127, bb_ix, s_ix >> 7, i_ix >> 7, s_ix & 127), 1)
    return A


def _pack(A, nj):
    """Pack nj counts per byte; plane j holds q = j*(QP//nj) + qq."""
    if nj == 1:
        # raw fp8 value bytes (exact for counts <= 448); scale 1.0
        P = A.astype(ml_dtypes.float8_e4m3).view(np.uint8)
        scale = 1.0
    else:
        bits = 8 // nj
        nqq = QP // nj
        Aj = A.reshape(128, BPC, NBLK, nj, nqq, 128)
        P = Aj[:, :, :, 0].astype(np.uint8).copy()
        for j in range(1, nj):
            P |= Aj[:, :, :, j] << (bits * j)
        scale = 512.0  # denormal ints decode as c * 2^-9
    return np.ascontiguousarray(P.reshape(128, -1)).view(np.uint16), scale


def _build_inputs(weights_core, mem_core, w, b, nj, scale):
    cpk, _ = _pack(_counts(mem_core), nj)

    wt = np.zeros((BPC, Q * 128), dtype=np.float32)
    wt[:, :N_ITEMS] = weights_core
    wt16 = (
        wt.reshape(BPC, Q, 128).transpose(2, 0, 1).reshape(128, BPC * Q)
    ).astype(np.float16)

    wbrow = np.empty((128, 2 * D), dtype=np.float32)
    wbrow[:, :D] = scale * w[None, :]
    wbrow[:, D:] = SET_SZ * b[None, :]
    return {"cpk": cpk, "wt16": wt16, "wbrow": wbrow}


def _choose_nj(membership):
    counts_max = 0
    for c in range(N_CORES):
        counts_max = max(
            counts_max, int(_counts(membership[c * BPC : (c + 1) * BPC]).max())
        )
    if counts_max <= 3:
        return 4
    if counts_max <= 7:
        return 2
    return 1


def kernel(weights, membership, w, b):
    weights = np.asarray(weights, dtype=np.float32)
    membership = np.asarray(membership)
    w = np.asarray(w, dtype=np.float32)
    b = np.asarray(b, dtype=np.float32)

    nj = _choose_nj(membership)
    key = f"nc{nj}"
    if key not in _CACHED:
        _CACHED[key] = _build_program(nj)
    nc = _CACHED[key]
    scale = 512.0 if nj > 1 else 1.0

    in_maps = [
        _build_inputs(
            weights[c * BPC : (c + 1) * BPC],
            membership[c * BPC : (c + 1) * BPC],
            w, b, nj, scale,
        )
        for c in range(N_CORES)
    ]
    res = run_bass_kernel_spmd(nc, in_maps, core_ids=list(range(N_CORES)))
    out = np.concatenate(
        [res.results[c]["out"] for c in range(N_CORES)], axis=0
    )
    return out.astype(np.float32)
